# revision 1
# baseline (speedup 1.0000x reference)
"""BartCustomAttention Trainium2 kernel.

Sharding: 8 cores = batch(2) x t-block(4, 256 rows each). Each core computes
all 16 heads for its 256 query rows; k/v projections for its batch element are
computed redundantly on the 4 cores sharing it (cheaper than cross-core
exchange at this size).

Relation-value term: out2[h,t,:] = sum_s attn[h,t,s] * E[r[t,s],:]
  = W[h,t,:41] @ E, with W the attn-weighted histogram of relation codes.
W is computed on the tensor engine against a host-precomputed one-hot(+ones)
fp8 matrix streamed from HBM. Per (4t-group, s-chunk-pair): 4 column-tiled
matmuls, each with a 32-col weight = (2 s-chunks x 16 heads) of exp'd scores
(the sc-pair stride-merges into one AP dim), against a [128, 84] two-block
one-hot. Even/odd-sc partial sums land in different psum rows/col-blocks and
are folded by a DVE add after the XBAR transpose (same partitions, different
columns). The ones column gives the softmax denominator Z per row; a small
fold-matmul sums the two half-Z's, and 1/Z is applied to rows 0:105 of the
output accumulator via per-head select-matmul broadcasts. W @ (E @ Wo_h.T)
folds into the output projection via a host-packed weight; bq folds into an
activation bias; bv and bo fold into the packed weight's ones-row.

Softmax skips the max-subtraction (scores are O(5) for this distribution;
mathematically identical).
"""

import sys

if "/opt/trn_rl_repo" not in sys.path:
    sys.path.insert(0, "/opt/trn_rl_repo")

import numpy as np
import ml_dtypes

import concourse.bass as bass
from concourse import bacc
import concourse.mybir as mybir
import concourse.tile as tile
from concourse import bass_utils

B, T, D, H, DH = 2, 1024, 1024, 16, 64
NJ = 42  # 41 relation bins + ones column (ones column = softmax Z)
TB = T // 4  # 256 query rows per core
P = 128
N_CORES = 8
NG = TB // 4  # 64 groups of 4 t's for the W-histogram phase

F32 = mybir.dt.float32
BF16 = mybir.dt.bfloat16
FP8 = mybir.dt.float8e4


def build_bass():
    nc = bacc.Bacc(None, target_bir_lowering=False)

    hsT = nc.dram_tensor("hsT", [D, T], BF16, kind="ExternalInput")
    hsTq = nc.dram_tensor("hsTq", [D, TB], BF16, kind="ExternalInput")
    WqT = nc.dram_tensor("WqT", [D, D], BF16, kind="ExternalInput")
    WkT = nc.dram_tensor("WkT", [D, D], BF16, kind="ExternalInput")
    WvT = nc.dram_tensor("WvT", [D, D], BF16, kind="ExternalInput")
    WoP = nc.dram_tensor("WoP", [H, P, D], BF16, kind="ExternalInput")
    bqk = nc.dram_tensor("bqk", [T, H], F32, kind="ExternalInput")
    OH = nc.dram_tensor("OH", [NG, P, 4 * 4 * 2 * NJ], FP8, kind="ExternalInput")
    hsel = nc.dram_tensor("hsel", [P, H * 105], F32, kind="ExternalInput")
    tmask = nc.dram_tensor("tmask", [P, TB], F32, kind="ExternalInput")
    fold = nc.dram_tensor("fold", [P, P], F32, kind="ExternalInput")
    out = nc.dram_tensor("out", [TB, D], F32, kind="ExternalOutput")

    with tile.TileContext(nc) as tc:
        with (
            tc.tile_pool(name="persist", bufs=1) as persist,
            tc.tile_pool(name="psProj", bufs=2, space="PSUM") as psProj,
            tc.tile_pool(name="psSc", bufs=2, space="PSUM") as psSc,
            tc.tile_pool(name="psO", bufs=2, space="PSUM") as psO,
            tc.tile_pool(name="psW", bufs=2, space="PSUM") as psW,
            tc.tile_pool(name="ohp", bufs=6) as ohp,
        ):
            # ---- persistent small inputs + big activations ----
            bqks = persist.tile([P, 8, H], F32)
            nc.gpsimd.dma_start(bqks[:], bqk.rearrange("(sc p) h -> p sc h", p=P))
            hsels = persist.tile([P, H, 105], F32)
            nc.gpsimd.dma_start(hsels[:], hsel.rearrange("p (h m) -> p h m", h=H))
            tmasks = persist.tile([P, TB], F32)
            nc.gpsimd.dma_start(tmasks[:], tmask[:, :])
            folds = persist.tile([P, P], F32)
            nc.gpsimd.dma_start(folds[:], fold[:, :])

            AT = persist.tile([P, 8, H, TB], BF16)  # [s, sc, h, t]
            oT = persist.tile([P, TB, H], BF16)
            Zh = persist.tile([P, NG], F32)  # half-Z at rows (32c+16ksc+h)
            RZ = persist.tile([P, NG], F32)  # 1/Z at rows (32c+h)
            nc.vector.memset(oT[96:128, :, :], 0.0)
            # row 105 is the constant-1 row that carries the bias through the
            # fused projection (WoP row 105 = bo + Wo@bv).
            onesbig = persist.tile([1, TB * H], BF16)
            nc.vector.memset(onesbig[:], 1.0)
            nc.gpsimd.dma_start(
                out=oT[105:106, :, :].rearrange("p t h -> p (t h)"),
                in_=onesbig[:],
            )

            with tc.tile_pool(name="acts", bufs=1) as acts:
                kT = acts.tile([P, 8, T], BF16)  # [d_model rows, ., s]
                vS = acts.tile([P, 8, D], BF16)  # [s rows, ., d_model]
                qT = acts.tile([P, 8, TB], BF16)

                with tc.tile_pool(name="win", bufs=1) as win:
                    # critical-path order on one queue: k-proj needs hsT+Wk
                    # first; everything else follows.
                    hsTs = win.tile([P, 8, T], BF16)
                    nc.sync.dma_start(
                        hsTs[:], hsT.rearrange("(ic p) s -> p ic s", p=P)
                    )
                    Wk_s = win.tile([P, 8, D], BF16)
                    nc.sync.dma_start(Wk_s[:], WkT.rearrange("(ic p) o -> p ic o", p=P))
                    Wv_s = win.tile([P, 8, D], BF16)
                    nc.sync.dma_start(Wv_s[:], WvT.rearrange("(ic p) o -> p ic o", p=P))
                    Wq_s = win.tile([P, 8, D], BF16)
                    nc.sync.dma_start(Wq_s[:], WqT.rearrange("(ic p) o -> p ic o", p=P))
                    hsTqs = win.tile([P, 8, TB], BF16)
                    nc.sync.dma_start(
                        hsTqs[:], hsTq.rearrange("(ic p) t -> p ic t", p=P)
                    )

                    # ---- phase 1: projections ----
                    for oc in range(8):
                        for n in range(2):
                            ps = psProj.tile([P, 512], F32, tag="proj")
                            for ic in range(8):
                                nc.tensor.matmul(
                                    ps[:],
                                    lhsT=Wk_s[:, ic, oc * P : (oc + 1) * P],
                                    rhs=hsTs[:, ic, n * 512 : (n + 1) * 512],
                                    start=(ic == 0),
                                    stop=(ic == 7),
                                    skip_group_check=True,
                                )
                            nc.vector.tensor_copy(
                                out=kT[:, oc, n * 512 : (n + 1) * 512], in_=ps[:]
                            )
                    for sc in range(8):
                        for n in range(2):
                            ps = psProj.tile([P, 512], F32, tag="proj")
                            for ic in range(8):
                                nc.tensor.matmul(
                                    ps[:],
                                    lhsT=hsTs[:, ic, sc * P : (sc + 1) * P],
                                    rhs=Wv_s[:, ic, n * 512 : (n + 1) * 512],
                                    start=(ic == 0),
                                    stop=(ic == 7),
                                    skip_group_check=True,
                                )
                            nc.vector.tensor_copy(
                                out=vS[:, sc, n * 512 : (n + 1) * 512], in_=ps[:]
                            )
                    for oc in range(8):
                        ps = psProj.tile([P, 512], F32, tag="proj")
                        for ic in range(8):
                            nc.tensor.matmul(
                                ps[:, :TB],
                                lhsT=Wq_s[:, ic, oc * P : (oc + 1) * P],
                                rhs=hsTqs[:, ic, :],
                                start=(ic == 0),
                                stop=(ic == 7),
                                skip_group_check=True,
                            )
                        nc.vector.tensor_copy(out=qT[:, oc, :], in_=ps[:, :TB])

                # ---- phase 2a+2b: scoresT -> exp -> AT; out1T ----
                for h in range(H):
                    base = (h % 2) * 64
                    oc = h // 2
                    for sc in range(8):
                        ps = psSc.tile([P, TB], F32, tag="scoresT")
                        nc.tensor.matmul(
                            ps[:],
                            lhsT=kT[base : base + 64, oc, sc * P : (sc + 1) * P],
                            rhs=qT[base : base + 64, oc, :],
                            start=True,
                            stop=True,
                            skip_group_check=True,
                        )
                        nc.scalar.activation(
                            AT[:, sc, h, :],
                            ps[:],
                            mybir.ActivationFunctionType.Exp,
                            bias=bqks[:, sc, h : h + 1],
                        )
                    pso = psO.tile([105, TB], F32, tag="out1T")
                    for sc in range(8):
                        nc.tensor.matmul(
                            pso[0:64, :],
                            lhsT=vS[:, sc, h * DH : (h + 1) * DH],
                            rhs=AT[:, sc, h, :],
                            start=(sc == 0),
                            stop=(sc == 7),
                            skip_group_check=True,
                        )
                    nc.vector.tensor_copy(out=oT[0:64, :, h], in_=pso[0:64, :])

            # acts (kT/vS/qT) freed here.
            with tc.tile_pool(name="late", bufs=1) as late:
                WoPs = late.tile([P, H, D], BF16)
                nc.sync.dma_start(WoPs[:], WoP.rearrange("h p o -> p h o"))
                WsbE = late.tile([P, NG, 64], BF16)  # even-sc W, j in 0:42
                WsbO = late.tile([P, NG, 64], BF16)  # odd-sc W
                WTe = late.tile([P, NG // 2, P], BF16)
                WTo = late.tile([P, NG // 2, P], BF16)
                Rzm = late.tile([P, TB], F32)
                nc.vector.memset(Rzm[:], 0.0)
                outsb = late.tile([P, 2, D], F32)

                # ---- phase 2c: W histogram matmuls ----
                # t = grp*4 + c; strip c: 32-col weight = (2 sc x 16 h) of t,
                # rhs = [O(sc even) | O(sc odd)] fp8. psw rows 32c+16ksc+h:
                # ksc=0 rows valid at cols 0:42, ksc=1 at 42:84.
                def assemble(gp):
                    for bb in range(2):
                        g = 2 * gp + bb
                        srcE = WTe[64 * bb : 64 * bb + 41, gp, :].rearrange(
                            "p (c x) -> p c x", c=4
                        )[:, :, 0:16]
                        srcO = WTo[64 * bb : 64 * bb + 41, gp, :].rearrange(
                            "p (c x) -> p c x", c=4
                        )[:, :, 16:32]
                        nc.vector.tensor_tensor(
                            out=oT[64:105, g * 4 : g * 4 + 4, :],
                            in0=srcE,
                            in1=srcO,
                            op=mybir.AluOpType.add,
                        )

                for grp in range(NG):
                    ohs = ohp.tile([P, 4, 2 * NJ * 4], FP8, tag="oh")
                    nc.gpsimd.dma_start(
                        ohs[:].rearrange("p a b -> p (a b)"), OH[grp, :, :]
                    )
                    psw = psW.tile([P, 2 * NJ], F32, tag="wps")
                    for sp in range(4):
                        for c in range(4):
                            nc.tensor.matmul(
                                psw[32 * c : 32 * c + 32, :],
                                lhsT=AT[
                                    :, 2 * sp : 2 * sp + 2, :, grp * 4 + c
                                ].rearrange("p a h -> p (a h)"),
                                rhs=ohs[:, sp, 2 * NJ * c : 2 * NJ * (c + 1)],
                                start=(sp == 0),
                                stop=(sp == 3),
                                tile_position=(0, 32 * c),
                                skip_group_check=True,
                            )
                    # ones column 41 = this row's half-Z
                    nc.vector.tensor_copy(out=Zh[:, grp : grp + 1], in_=psw[:, 41:42])
                    nc.vector.tensor_copy(out=WsbE[:, grp, 0:NJ], in_=psw[:, 0:NJ])
                    nc.vector.tensor_copy(
                        out=WsbO[:, grp, 0:NJ], in_=psw[:, NJ : 2 * NJ]
                    )
                    if grp % 2 == 1:
                        gp = grp // 2
                        nc.sync.dma_start_transpose(
                            WTe[:, gp, :],
                            WsbE[:, grp - 1 : grp + 1, :].rearrange(
                                "p a x -> p (a x)"
                            ),
                        )
                        nc.sync.dma_start_transpose(
                            WTo[:, gp, :],
                            WsbO[:, grp - 1 : grp + 1, :].rearrange(
                                "p a x -> p (a x)"
                            ),
                        )
                    # assemble with a 4-pair delay so transposes are done
                    if grp % 2 == 1 and grp // 2 >= 4:
                        assemble(grp // 2 - 4)
                for gp in range(NG // 2 - 4, NG // 2):
                    assemble(gp)

                # ---- phase 2g: Z fold + 1/Z broadcast + normalize ----
                psz = psW.tile([P, 2 * NJ], F32, tag="wps")
                nc.tensor.matmul(
                    psz[:, 0:NG],
                    lhsT=folds[:],
                    rhs=Zh[:],
                    start=True,
                    stop=True,
                    skip_group_check=True,
                )
                nc.vector.reciprocal(out=RZ[:], in_=psz[:, 0:NG])
                # Rzm[p, t] = 1/Z[t, h(p)] at valid rows; junk rows stay 0.
                for c in range(4):
                    nc.vector.tensor_tensor(
                        out=Rzm[32 * c : 32 * c + 16, :].rearrange(
                            "p (g u) -> p g u", g=NG
                        ),
                        in0=RZ[32 * c : 32 * c + 16, :, None].to_broadcast(
                            [16, NG, 4]
                        ),
                        in1=tmasks[32 * c : 32 * c + 16, :].rearrange(
                            "p (g u) -> p g u", g=NG
                        ),
                        op=mybir.AluOpType.mult,
                    )
                for h in range(H):
                    psb = psO.tile([105, TB], F32, tag="out1T")
                    nc.tensor.matmul(
                        psb[:],
                        lhsT=hsels[:, h, :],
                        rhs=Rzm[:],
                        start=True,
                        stop=True,
                        skip_group_check=True,
                    )
                    nc.vector.tensor_tensor(
                        out=oT[0:105, :, h],
                        in0=oT[0:105, :, h],
                        in1=psb[:],
                        op=mybir.AluOpType.mult,
                    )

                # ---- phase 3: fused output projection ----
                for tc_i in range(2):
                    for ocj in range(2):
                        pso = psProj.tile([P, 512], F32, tag="proj")
                        for h in range(H):
                            nc.tensor.matmul(
                                pso[:],
                                lhsT=oT[:, tc_i * P : (tc_i + 1) * P, h],
                                rhs=WoPs[:, h, ocj * 512 : (ocj + 1) * 512],
                                start=(h == 0),
                                stop=(h == H - 1),
                                skip_group_check=True,
                            )
                        nc.vector.tensor_copy(
                            out=outsb[:, tc_i, ocj * 512 : (ocj + 1) * 512], in_=pso[:]
                        )
                nc.sync.dma_start(
                    out=out.rearrange("(tc p) o -> p tc o", p=P), in_=outsb[:]
                )

    nc.compile()
    return nc


_NC = None
_last_in_maps = None


def _get_nc():
    global _NC
    if _NC is None:
        _NC = build_bass()
    return _NC


def _prep_in_maps(hidden_states, relation_inputs, Wq, bq, Wk, bk, Wv, bv, Wo, bo, rel_emb):
    hidden_states = np.asarray(hidden_states, dtype=np.float32)
    relation_inputs = np.asarray(relation_inputs)
    scale = DH ** -0.5
    bf = ml_dtypes.bfloat16
    fp8np = mybir.dt.np(FP8)

    WqTs = (np.asarray(Wq, np.float32).T * scale).astype(bf)
    WkT = np.asarray(Wk, np.float32).T.astype(bf)
    WvT = np.asarray(Wv, np.float32).T.astype(bf)
    Wo = np.asarray(Wo, np.float32)
    E = np.asarray(rel_emb, np.float32)

    WoP = np.zeros((H, P, D), np.float32)
    for h in range(H):
        Wo_h = Wo[:, h * DH : (h + 1) * DH]  # [D, 64]
        WoP[h, 0:64, :] = Wo_h.T
        WoP[h, 64:105, :] = E @ Wo_h.T
    WoP[0, 105, :] = np.asarray(bo, np.float32) + Wo @ np.asarray(bv, np.float32)
    WoP = WoP.astype(bf)

    # bqk[s, h] = k_h[s] . (bq_h * scale) = (hs_b @ Wk_h.T @ bq_h*scale)[s]
    bqs = np.asarray(bq, np.float32) * scale
    wb = np.zeros((D, H), np.float32)
    for h in range(H):
        wb[:, h] = np.asarray(Wk, np.float32)[h * DH : (h + 1) * DH, :].T @ bqs[
            h * DH : (h + 1) * DH
        ]

    # helpers: p = 32c + 16ksc + hh in the W-phase psum layout
    pidx = np.arange(P)
    c_p, h_p = pidx // 32, pidx % 32
    hsel_np = (h_p[:, None] == np.arange(H)[None, :]).astype(np.float32)
    hsel_np = np.repeat(hsel_np[:, :, None], 105, axis=2).reshape(P, H * 105)
    tt = np.arange(TB)
    tmask_np = (tt[None, :] % 4 == c_p[:, None]).astype(np.float32)
    fold_np = np.zeros((P, P), np.float32)
    fold_np[pidx, 32 * (pidx // 32) + (pidx % 16)] = 1.0

    in_maps = []
    for core in range(N_CORES):
        b, tb = core // 4, core % 4
        hs_b = hidden_states[b]
        hsT_b = np.ascontiguousarray(hs_b.T).astype(bf)
        hsTq = np.ascontiguousarray(hs_b.T[:, tb * TB : (tb + 1) * TB]).astype(bf)
        bqk_c = (hs_b @ wb).astype(np.float32)

        # one-hot(+ones) blocks: OH[g, s', (sp, c, ke, j)]; t = g*4 + c,
        # sc = 2*sp + ke
        rc = np.asarray(relation_inputs[b, tb * TB : (tb + 1) * TB, :])  # [256,1024]
        oh = np.zeros((TB * T, NJ), np.uint8)
        oh[np.arange(TB * T), rc.ravel()] = 1
        oh = oh.reshape(TB, T, NJ)
        oh[:, :, 41] = 1
        oh = oh.reshape(NG, 4, 4, 2, P, NJ).transpose(0, 4, 2, 1, 3, 5)
        oh = np.ascontiguousarray(oh).reshape(NG, P, 4 * 4 * 2 * NJ).astype(fp8np)

        in_maps.append(
            dict(
                hsT=hsT_b,
                hsTq=hsTq,
                WqT=WqTs,
                WkT=WkT,
                WvT=WvT,
                WoP=WoP,
                bqk=bqk_c,
                OH=oh,
                hsel=hsel_np,
                tmask=tmask_np,
                fold=fold_np,
            )
        )
    return in_maps


def kernel(hidden_states, relation_inputs, Wq, bq, Wk, bk, Wv, bv, Wo, bo, rel_emb):
    global _last_in_maps
    in_maps = _prep_in_maps(
        hidden_states, relation_inputs, Wq, bq, Wk, bk, Wv, bv, Wo, bo, rel_emb
    )
    _last_in_maps = in_maps
    nc = _get_nc()
    res = bass_utils.run_bass_kernel_spmd(nc, in_maps, core_ids=list(range(N_CORES)))
    outs = [np.asarray(r["out"], np.float32) for r in res.results]
    full = np.empty((B, T, D), np.float32)
    for core in range(N_CORES):
        b, tb = core // 4, core % 4
        full[b, tb * TB : (tb + 1) * TB, :] = outs[core]
    return full



# revision 3
# speedup vs baseline: 1.6375x; 1.6375x over previous
"""BartCustomAttention Trainium2 kernel (v2).

Sharding: 8 cores = batch(2) x t-block(4, 256 rows each). Each core computes
all 16 heads for its 256 query rows; k/v projections for its batch element are
computed redundantly on the 4 cores sharing it.

Relation-value term: out2[h,t,:] = sum_s attn[h,t,s] * E[r[t,s],:]
  = W[h,t,:41] @ E, with W the attn-weighted histogram of relation codes,
computed on the tensor engine against a host-precomputed one-hot(+ones)
fp8 matrix streamed from HBM (ones column = softmax denominator Z).
W @ (E @ Wo_h.T) folds into the output projection via a host-packed weight;
bq folds into an activation bias; bv and bo fold into the packed weight's
ones-row. Softmax skips the max-subtraction (scores are O(5)).

v2 changes vs v1 (391926 ns):
- W-phase pair transposes moved from sync-queue XBAR DMA (1.2us each,
  serialized, xbar-mode switching against other DMAs) to PE-array
  transposes into PSUM; the even/odd fold (assemble) reads the transposed
  tiles straight from PSUM.
- Input DMAs split into per-oc / per-(ic,half) chunks across queues in
  consumption order; q-projection weights land first so the PE starts
  ~1.5us in and never sees a >3.4us gap (keeps the HAM clock gate at
  2.4GHz instead of the 1.2GHz cold state the v1 trace showed 72% of
  the time).
- Phase order qproj -> kproj -> (scores+exp interleaved with vproj) ->
  out1 -> W -> norm -> oproj; the scalar-engine exps hide under the
  vproj matmul stream.
- Output projection DMA'd per 512-col chunk.
"""

import sys

if "/opt/trn_rl_repo" not in sys.path:
    sys.path.insert(0, "/opt/trn_rl_repo")

import numpy as np
import ml_dtypes

import concourse.bass as bass
from concourse import bacc
import concourse.mybir as mybir
import concourse.tile as tile
from concourse import bass_utils

B, T, D, H, DH = 2, 1024, 1024, 16, 64
NJ = 42  # 41 relation bins + ones column (ones column = softmax Z)
TB = T // 4  # 256 query rows per core
P = 128
N_CORES = 8
NG = TB // 4  # 64 groups of 4 t's for the W-histogram phase

F32 = mybir.dt.float32
BF16 = mybir.dt.bfloat16
FP8 = mybir.dt.float8e4


def build_bass():
    nc = bacc.Bacc(None, target_bir_lowering=False)

    hsT = nc.dram_tensor("hsT", [D, T], BF16, kind="ExternalInput")
    hsTq = nc.dram_tensor("hsTq", [D, TB], BF16, kind="ExternalInput")
    WqP = nc.dram_tensor("WqP", [8, P, D], BF16, kind="ExternalInput")
    WkP = nc.dram_tensor("WkP", [8, P, D], BF16, kind="ExternalInput")
    WvT = nc.dram_tensor("WvT", [D, D], BF16, kind="ExternalInput")
    WoP = nc.dram_tensor("WoP", [H, P, D], BF16, kind="ExternalInput")
    bqk = nc.dram_tensor("bqk", [T, H], F32, kind="ExternalInput")
    OH = nc.dram_tensor("OH", [NG, P, 4 * 4 * 2 * NJ], FP8, kind="ExternalInput")
    hsel = nc.dram_tensor("hsel", [P, H * 105], F32, kind="ExternalInput")
    tmask = nc.dram_tensor("tmask", [P, TB], F32, kind="ExternalInput")
    fold = nc.dram_tensor("fold", [P, P], F32, kind="ExternalInput")
    ident = nc.dram_tensor("ident", [P, P], BF16, kind="ExternalInput")
    out = nc.dram_tensor("out", [TB, D], F32, kind="ExternalOutput")

    with tile.TileContext(nc) as tc:
        with (
            tc.tile_pool(name="persist", bufs=1) as persist,
            tc.tile_pool(name="psProj", bufs=2, space="PSUM") as psProj,
            tc.tile_pool(name="psW", bufs=2, space="PSUM") as psW,
            tc.tile_pool(name="ohp", bufs=6) as ohp,
        ):
            # ---- persistent small inputs (scalar queue; all tiny) ----
            bqks = persist.tile([P, 8, H], F32)
            nc.scalar.dma_start(bqks[:], bqk.rearrange("(sc p) h -> p sc h", p=P))
            hsels = persist.tile([P, H, 105], F32)
            nc.scalar.dma_start(hsels[:], hsel.rearrange("p (h m) -> p h m", h=H))
            tmasks = persist.tile([P, TB], F32)
            nc.scalar.dma_start(tmasks[:], tmask[:, :])
            folds = persist.tile([P, P], F32)
            nc.scalar.dma_start(folds[:], fold[:, :])
            idents = persist.tile([P, P], BF16)
            nc.scalar.dma_start(idents[:], ident[:, :])

            AT = persist.tile([P, 8, H, TB], BF16)  # [s, sc, h, t]
            oT = persist.tile([P, TB, H], BF16)
            Zh = persist.tile([P, NG], F32)  # half-Z at rows (32c+16ksc+h)
            RZ = persist.tile([P, NG], F32)  # 1/Z at rows (32c+h)
            nc.vector.memset(oT[96:128, :, :], 0.0)
            # row 105 is the constant-1 row that carries the bias through the
            # fused projection (WoP row 105 = bo + Wo@bv).
            onesbig = persist.tile([1, TB * H], BF16)
            nc.vector.memset(onesbig[:], 1.0)
            nc.gpsimd.dma_start(
                out=oT[105:106, :, :].rearrange("p t h -> p (t h)"),
                in_=onesbig[:],
            )

            with tc.tile_pool(name="acts", bufs=1) as acts:
                kT = acts.tile([P, 8, T], BF16)  # [d_model rows, oc, s]
                vS = acts.tile([P, 8, D], BF16)  # [s rows, sc, d_model]
                qT = acts.tile([P, 8, TB], BF16)

                with (
                    tc.tile_pool(name="psSc", bufs=2, space="PSUM") as psSc,
                    tc.tile_pool(name="psO", bufs=2, space="PSUM") as psO,
                ):
                    with tc.tile_pool(name="win", bufs=1) as win:
                        # DMA in consumption order. sync: hsTq then hsT
                        # (per-(ic,half) chunks, n0 halves first so kproj n0
                        # groups can close early). gpsimd: WqP ocs, WkP ocs,
                        # bqk-free, WvT ics.
                        hsTqs = win.tile([P, 8, TB], BF16)
                        for ic in range(8):
                            nc.sync.dma_start(
                                hsTqs[:, ic, :],
                                hsTq.rearrange("(ic p) t -> p ic t", p=P)[:, ic, :],
                            )
                        Wq_s = win.tile([P, 8, D], BF16)  # [p, oc, ic*128+j]
                        for oc in range(8):
                            nc.gpsimd.dma_start(Wq_s[:, oc, :], WqP[oc, :, :])
                        hsTs = win.tile([P, 8, T], BF16)  # [p, ic, s]
                        for n in range(2):
                            for ic in range(8):
                                nc.sync.dma_start(
                                    hsTs[:, ic, n * 512 : (n + 1) * 512],
                                    hsT.rearrange("(ic p) s -> p ic s", p=P)[
                                        :, ic, n * 512 : (n + 1) * 512
                                    ],
                                )
                        Wk_s = win.tile([P, 8, D], BF16)  # [p, oc, ic*128+j]
                        for oc in range(8):
                            nc.gpsimd.dma_start(Wk_s[:, oc, :], WkP[oc, :, :])
                        Wv_s = win.tile([P, 8, D], BF16)  # [p, ic, o]
                        for ic in range(8):
                            nc.gpsimd.dma_start(
                                Wv_s[:, ic, :],
                                WvT.rearrange("(ic p) o -> p ic o", p=P)[:, ic, :],
                            )

                        # ---- q projection ----
                        for oc in range(8):
                            ps = psProj.tile([P, 512], F32, tag="proj")
                            for ic in range(8):
                                nc.tensor.matmul(
                                    ps[:, :TB],
                                    lhsT=Wq_s[:, oc, ic * P : (ic + 1) * P],
                                    rhs=hsTqs[:, ic, :],
                                    start=(ic == 0),
                                    stop=(ic == 7),
                                    skip_group_check=True,
                                )
                            nc.vector.tensor_copy(out=qT[:, oc, :], in_=ps[:, :TB])

                        # ---- k projection (n-outer so n0 groups only need
                        # the n0 halves of hsT) ----
                        for n in range(2):
                            for oc in range(8):
                                ps = psProj.tile([P, 512], F32, tag="proj")
                                for ic in range(8):
                                    nc.tensor.matmul(
                                        ps[:],
                                        lhsT=Wk_s[:, oc, ic * P : (ic + 1) * P],
                                        rhs=hsTs[:, ic, n * 512 : (n + 1) * 512],
                                        start=(ic == 0),
                                        stop=(ic == 7),
                                        skip_group_check=True,
                                    )
                                nc.vector.tensor_copy(
                                    out=kT[:, oc, n * 512 : (n + 1) * 512], in_=ps[:]
                                )

                        # ---- scores+exp interleaved with v projection ----
                        # exp on scalar takes ~333ns per (h,sc) vs ~110ns for
                        # the score matmul; interleaving two vproj psum
                        # groups per head keeps the PE streaming while the
                        # scalar engine drains the score psum slots.
                        vgroups = [(sc, n) for sc in range(8) for n in range(2)]
                        for h in range(H):
                            base = (h % 2) * 64
                            oc = h // 2
                            for sc in range(8):
                                ps = psSc.tile([P, TB], F32, tag="scoresT")
                                nc.tensor.matmul(
                                    ps[:],
                                    lhsT=kT[base : base + 64, oc, sc * P : (sc + 1) * P],
                                    rhs=qT[base : base + 64, oc, :],
                                    start=True,
                                    stop=True,
                                    skip_group_check=True,
                                )
                                nc.scalar.activation(
                                    AT[:, sc, h, :],
                                    ps[:],
                                    mybir.ActivationFunctionType.Exp,
                                    bias=bqks[:, sc, h : h + 1],
                                )
                            for sc, n in vgroups[2 * h : 2 * h + 2]:
                                ps = psProj.tile([P, 512], F32, tag="proj")
                                for ic in range(8):
                                    nc.tensor.matmul(
                                        ps[:],
                                        lhsT=hsTs[:, ic, sc * P : (sc + 1) * P],
                                        rhs=Wv_s[:, ic, n * 512 : (n + 1) * 512],
                                        start=(ic == 0),
                                        stop=(ic == 7),
                                        skip_group_check=True,
                                    )
                                nc.vector.tensor_copy(
                                    out=vS[:, sc, n * 512 : (n + 1) * 512], in_=ps[:]
                                )

                    # ---- out1: attn @ v per head ----
                    for h in range(H):
                        pso = psO.tile([64, TB], F32, tag="out1T")
                        for sc in range(8):
                            nc.tensor.matmul(
                                pso[:],
                                lhsT=vS[:, sc, h * DH : (h + 1) * DH],
                                rhs=AT[:, sc, h, :],
                                start=(sc == 0),
                                stop=(sc == 7),
                                skip_group_check=True,
                            )
                        nc.vector.tensor_copy(out=oT[0:64, :, h], in_=pso[:])

            # acts (kT/vS/qT) freed here; psSc/psO banks freed for psL.
            with (
                tc.tile_pool(name="late", bufs=1) as late,
                tc.tile_pool(name="psL", bufs=2, space="PSUM") as psL,
            ):
                WoPs = late.tile([P, H, D], BF16)
                nc.sync.dma_start(WoPs[:], WoP.rearrange("h p o -> p h o"))
                WsbE = late.tile([P, NG, 64], BF16)  # even-sc W, j in 0:42
                WsbO = late.tile([P, NG, 64], BF16)  # odd-sc W
                Rzm = late.tile([P, TB], F32)
                nc.vector.memset(Rzm[:], 0.0)
                outsb = late.tile([P, 2, D], F32)

                # ---- W histogram matmuls ----
                # t = grp*4 + c; strip c: 32-col weight = (2 sc x 16 h) of t,
                # rhs = [O(sc even) | O(sc odd)] fp8. psw rows 32c+16ksc+h:
                # ksc=0 rows valid at cols 0:42, ksc=1 at 42:84.
                for grp in range(NG):
                    ohs = ohp.tile([P, 4, 2 * NJ * 4], FP8, tag="oh")
                    nc.gpsimd.dma_start(
                        ohs[:].rearrange("p a b -> p (a b)"), OH[grp, :, :]
                    )
                    psw = psW.tile([P, 2 * NJ], F32, tag="wps")
                    for sp in range(4):
                        for c in range(4):
                            nc.tensor.matmul(
                                psw[32 * c : 32 * c + 32, :],
                                lhsT=AT[
                                    :, 2 * sp : 2 * sp + 2, :, grp * 4 + c
                                ].rearrange("p a h -> p (a h)"),
                                rhs=ohs[:, sp, 2 * NJ * c : 2 * NJ * (c + 1)],
                                start=(sp == 0),
                                stop=(sp == 3),
                                tile_position=(0, 32 * c),
                                skip_group_check=True,
                            )
                    # ones column 41 = this row's half-Z
                    nc.vector.tensor_copy(out=Zh[:, grp : grp + 1], in_=psw[:, 41:42])
                    nc.vector.tensor_copy(out=WsbE[:, grp, 0:NJ], in_=psw[:, 0:NJ])
                    nc.vector.tensor_copy(
                        out=WsbO[:, grp, 0:NJ], in_=psw[:, NJ : 2 * NJ]
                    )
                    if grp % 2 == 1:
                        gp = grp // 2
                        psTE = psL.tile([P, P], BF16, tag="tE")
                        nc.tensor.transpose(
                            psTE[:],
                            WsbE[:, grp - 1 : grp + 1, :].rearrange("p a x -> p (a x)"),
                            idents[:],
                        )
                        psTO = psL.tile([P, P], BF16, tag="tO")
                        nc.tensor.transpose(
                            psTO[:],
                            WsbO[:, grp - 1 : grp + 1, :].rearrange("p a x -> p (a x)"),
                            idents[:],
                        )
                        # DVE can read only ONE psum input per op: copy the
                        # even part into oT, then add the odd part in place.
                        for bb in range(2):
                            g = 2 * gp + bb
                            srcE = psTE[64 * bb : 64 * bb + 41, :].rearrange(
                                "p (c x) -> p c x", c=4
                            )[:, :, 0:16]
                            srcO = psTO[64 * bb : 64 * bb + 41, :].rearrange(
                                "p (c x) -> p c x", c=4
                            )[:, :, 16:32]
                            nc.vector.tensor_copy(
                                out=oT[64:105, g * 4 : g * 4 + 4, :], in_=srcE
                            )
                            nc.vector.tensor_tensor(
                                out=oT[64:105, g * 4 : g * 4 + 4, :],
                                in0=oT[64:105, g * 4 : g * 4 + 4, :],
                                in1=srcO,
                                op=mybir.AluOpType.add,
                            )

                # ---- Z fold + 1/Z broadcast + normalize ----
                psz = psW.tile([P, 2 * NJ], F32, tag="wps")
                nc.tensor.matmul(
                    psz[:, 0:NG],
                    lhsT=folds[:],
                    rhs=Zh[:],
                    start=True,
                    stop=True,
                    skip_group_check=True,
                )
                nc.vector.reciprocal(out=RZ[:], in_=psz[:, 0:NG])
                # Rzm[p, t] = 1/Z[t, h(p)] at valid rows; junk rows stay 0.
                for c in range(4):
                    nc.vector.tensor_tensor(
                        out=Rzm[32 * c : 32 * c + 16, :].rearrange(
                            "p (g u) -> p g u", g=NG
                        ),
                        in0=RZ[32 * c : 32 * c + 16, :, None].to_broadcast(
                            [16, NG, 4]
                        ),
                        in1=tmasks[32 * c : 32 * c + 16, :].rearrange(
                            "p (g u) -> p g u", g=NG
                        ),
                        op=mybir.AluOpType.mult,
                    )
                for h in range(H):
                    psb = psL.tile([105, TB], F32, tag="tE")
                    nc.tensor.matmul(
                        psb[:],
                        lhsT=hsels[:, h, :],
                        rhs=Rzm[:],
                        start=True,
                        stop=True,
                        skip_group_check=True,
                    )
                    nc.vector.tensor_tensor(
                        out=oT[0:105, :, h],
                        in0=oT[0:105, :, h],
                        in1=psb[:],
                        op=mybir.AluOpType.mult,
                    )

                # ---- fused output projection, DMA'd per chunk ----
                outv = out.rearrange("(tc p) o -> p tc o", p=P)
                for tc_i in range(2):
                    for ocj in range(2):
                        pso = psProj.tile([P, 512], F32, tag="proj")
                        for h in range(H):
                            nc.tensor.matmul(
                                pso[:],
                                lhsT=oT[:, tc_i * P : (tc_i + 1) * P, h],
                                rhs=WoPs[:, h, ocj * 512 : (ocj + 1) * 512],
                                start=(h == 0),
                                stop=(h == H - 1),
                                skip_group_check=True,
                            )
                        nc.vector.tensor_copy(
                            out=outsb[:, tc_i, ocj * 512 : (ocj + 1) * 512], in_=pso[:]
                        )
                        nc.sync.dma_start(
                            out=outv[:, tc_i, ocj * 512 : (ocj + 1) * 512],
                            in_=outsb[:, tc_i, ocj * 512 : (ocj + 1) * 512],
                        )

    nc.compile()
    return nc


_NC = None
_last_in_maps = None


def _get_nc():
    global _NC
    if _NC is None:
        _NC = build_bass()
    return _NC


def _prep_in_maps(hidden_states, relation_inputs, Wq, bq, Wk, bk, Wv, bv, Wo, bo, rel_emb):
    hidden_states = np.asarray(hidden_states, dtype=np.float32)
    relation_inputs = np.asarray(relation_inputs)
    scale = DH ** -0.5
    bf = ml_dtypes.bfloat16
    fp8np = mybir.dt.np(FP8)

    def per_oc(wT):  # [in, out] -> [oc, p, ic*128+j]
        return np.ascontiguousarray(
            wT.reshape(8, P, 8, P).transpose(2, 1, 0, 3).reshape(8, P, D)
        )

    WqP = per_oc((np.asarray(Wq, np.float32).T * scale)).astype(bf)
    WkP = per_oc(np.asarray(Wk, np.float32).T).astype(bf)
    WvT = np.asarray(Wv, np.float32).T.astype(bf)
    Wo = np.asarray(Wo, np.float32)
    E = np.asarray(rel_emb, np.float32)

    WoP = np.zeros((H, P, D), np.float32)
    for h in range(H):
        Wo_h = Wo[:, h * DH : (h + 1) * DH]  # [D, 64]
        WoP[h, 0:64, :] = Wo_h.T
        WoP[h, 64:105, :] = E @ Wo_h.T
    WoP[0, 105, :] = np.asarray(bo, np.float32) + Wo @ np.asarray(bv, np.float32)
    WoP = WoP.astype(bf)

    # bqk[s, h] = k_h[s] . (bq_h * scale) = (hs_b @ Wk_h.T @ bq_h*scale)[s]
    bqs = np.asarray(bq, np.float32) * scale
    wb = np.zeros((D, H), np.float32)
    for h in range(H):
        wb[:, h] = np.asarray(Wk, np.float32)[h * DH : (h + 1) * DH, :].T @ bqs[
            h * DH : (h + 1) * DH
        ]

    # helpers: p = 32c + 16ksc + hh in the W-phase psum layout
    pidx = np.arange(P)
    c_p, h_p = pidx // 32, pidx % 32
    hsel_np = (h_p[:, None] == np.arange(H)[None, :]).astype(np.float32)
    hsel_np = np.repeat(hsel_np[:, :, None], 105, axis=2).reshape(P, H * 105)
    tt = np.arange(TB)
    tmask_np = (tt[None, :] % 4 == c_p[:, None]).astype(np.float32)
    fold_np = np.zeros((P, P), np.float32)
    fold_np[pidx, 32 * (pidx // 32) + (pidx % 16)] = 1.0
    ident_np = np.eye(P, dtype=np.float32).astype(bf)

    in_maps = []
    for core in range(N_CORES):
        b, tb = core // 4, core % 4
        hs_b = hidden_states[b]
        hsT_b = np.ascontiguousarray(hs_b.T).astype(bf)
        hsTq = np.ascontiguousarray(hs_b.T[:, tb * TB : (tb + 1) * TB]).astype(bf)
        bqk_c = (hs_b @ wb).astype(np.float32)

        # one-hot(+ones) blocks: OH[g, s', (sp, c, ke, j)]; t = g*4 + c,
        # sc = 2*sp + ke
        rc = np.asarray(relation_inputs[b, tb * TB : (tb + 1) * TB, :])  # [256,1024]
        oh = np.zeros((TB * T, NJ), np.uint8)
        oh[np.arange(TB * T), rc.ravel()] = 1
        oh = oh.reshape(TB, T, NJ)
        oh[:, :, 41] = 1
        oh = oh.reshape(NG, 4, 4, 2, P, NJ).transpose(0, 4, 2, 1, 3, 5)
        oh = np.ascontiguousarray(oh).reshape(NG, P, 4 * 4 * 2 * NJ).astype(fp8np)

        in_maps.append(
            dict(
                hsT=hsT_b,
                hsTq=hsTq,
                WqP=WqP,
                WkP=WkP,
                WvT=WvT,
                WoP=WoP,
                bqk=bqk_c,
                OH=oh,
                hsel=hsel_np,
                tmask=tmask_np,
                fold=fold_np,
                ident=ident_np,
            )
        )
    return in_maps


def kernel(hidden_states, relation_inputs, Wq, bq, Wk, bk, Wv, bv, Wo, bo, rel_emb):
    global _last_in_maps
    in_maps = _prep_in_maps(
        hidden_states, relation_inputs, Wq, bq, Wk, bk, Wv, bv, Wo, bo, rel_emb
    )
    _last_in_maps = in_maps
    nc = _get_nc()
    res = bass_utils.run_bass_kernel_spmd(nc, in_maps, core_ids=list(range(N_CORES)))
    outs = [np.asarray(r["out"], np.float32) for r in res.results]
    full = np.empty((B, T, D), np.float32)
    for core in range(N_CORES):
        b, tb = core // 4, core % 4
        full[b, tb * TB : (tb + 1) * TB, :] = outs[core]
    return full


# revision 4
# speedup vs baseline: 1.8570x; 1.1340x over previous
"""BartCustomAttention Trainium2 kernel (v3).

Sharding: 8 cores = batch(2) x t-block(4, 256 rows each). Each core computes
all 16 heads for its 256 query rows; k/v projections for its batch element are
computed redundantly on the 4 cores sharing it.

Relation-value term: out2[h,t,:] = sum_s attn[h,t,s] * E[r[t,s],:]
  = W[h,t,:41] @ E, with W the attn-weighted histogram of relation codes,
computed on the tensor engine against a host-precomputed one-hot(+ones)
fp8 matrix streamed from HBM (ones column = softmax denominator Z).
W @ (E @ Wo_h.T) folds into the output projection via a host-packed weight;
bv and bo fold into the packed weight's ones-row. Softmax skips the
max-subtraction (scores are O(5)).

v3 changes vs v2 (277us) / v1 (392us):
- oT layout [P, H, TB] so the per-head normalize multiplies / out1 copies
  are contiguous on DVE (v2 paid 1.3us per strided [105,TB] multiply).
- bq == 0 for this problem, so the exp bias path is dropped (a with_bias
  build variant keeps generality) and exps are batched 4 score-chunks per
  ACTIVATE: 32 ops instead of 128 cuts scalar time under the PE's
  scores+vproj window, which in v2 backpressured the PE enough that the
  HAM clock gate dropped to 1.2GHz for the whole W phase.
- Identity-transpose warmup matmuls at t~5us warm the HAM gate during the
  input DMA lead-in.
- OH one-hot tiles alternate sync/gpsimd queues (one queue's ~0.7us
  dispatch per tile couldn't feed the warm-clock W phase); WoP moves to
  the scalar queue.
- PSUM pools rescoped: scores get 2x[128,1024] tiles; psW/psL open after
  the projection-phase pools close.
"""

import sys

if "/opt/trn_rl_repo" not in sys.path:
    sys.path.insert(0, "/opt/trn_rl_repo")

import numpy as np
import ml_dtypes

import concourse.bass as bass
from concourse import bacc
import concourse.mybir as mybir
import concourse.tile as tile
from concourse import bass_utils

B, T, D, H, DH = 2, 1024, 1024, 16, 64
NJ = 42  # 41 relation bins + ones column (ones column = softmax Z)
TB = T // 4  # 256 query rows per core
P = 128
N_CORES = 8
NG = TB // 4  # 64 groups of 4 t's for the W-histogram phase
N_WARM = 30

F32 = mybir.dt.float32
BF16 = mybir.dt.bfloat16
FP8 = mybir.dt.float8e4


def build_bass(with_bias: bool):
    nc = bacc.Bacc(None, target_bir_lowering=False)

    hsT = nc.dram_tensor("hsT", [D, T], BF16, kind="ExternalInput")
    hsTq = nc.dram_tensor("hsTq", [D, TB], BF16, kind="ExternalInput")
    WqP = nc.dram_tensor("WqP", [8, P, D], BF16, kind="ExternalInput")
    WkP = nc.dram_tensor("WkP", [8, P, D], BF16, kind="ExternalInput")
    WvT = nc.dram_tensor("WvT", [D, D], BF16, kind="ExternalInput")
    WoP = nc.dram_tensor("WoP", [H, P, D], BF16, kind="ExternalInput")
    bqk = nc.dram_tensor("bqk", [T, H], F32, kind="ExternalInput")
    OH = nc.dram_tensor("OH", [NG, P, 4 * 4 * 2 * NJ], FP8, kind="ExternalInput")
    hsel = nc.dram_tensor("hsel", [P, H * 105], F32, kind="ExternalInput")
    tmask = nc.dram_tensor("tmask", [P, TB], F32, kind="ExternalInput")
    fold = nc.dram_tensor("fold", [P, P], F32, kind="ExternalInput")
    ident = nc.dram_tensor("ident", [P, P], BF16, kind="ExternalInput")
    out = nc.dram_tensor("out", [TB, D], F32, kind="ExternalOutput")

    with tile.TileContext(nc) as tc:
        with (
            tc.tile_pool(name="persist", bufs=1) as persist,
            tc.tile_pool(name="psProj", bufs=2, space="PSUM") as psProj,
            tc.tile_pool(name="ohp", bufs=8) as ohp,
        ):
            # ---- small inputs on the scalar queue; ident first so the
            # HAM warmup matmuls can start during the big-input DMA ----
            idents = persist.tile([P, P], BF16)
            nc.scalar.dma_start(idents[:], ident[:, :])
            hsels = persist.tile([P, H, 105], F32)
            nc.scalar.dma_start(hsels[:], hsel.rearrange("p (h m) -> p h m", h=H))
            tmasks = persist.tile([P, TB], F32)
            nc.scalar.dma_start(tmasks[:], tmask[:, :])
            folds = persist.tile([P, P], F32)
            nc.scalar.dma_start(folds[:], fold[:, :])
            if with_bias:
                bqks = persist.tile([P, 8, H], F32)
                nc.scalar.dma_start(
                    bqks[:], bqk.rearrange("(sc p) h -> p sc h", p=P)
                )

            AT = persist.tile([P, 8, H, TB], BF16)  # [s, sc, h, t]
            oT = persist.tile([P, H, TB], BF16)  # [row, h, t]
            Zh = persist.tile([P, NG], F32)  # half-Z at rows (32c+16ksc+h)
            RZ = persist.tile([P, NG], F32)  # 1/Z at rows (32c+h)
            nc.vector.memset(oT[96:128, :, :], 0.0)
            # row 105 is the constant-1 row that carries the bias through the
            # fused projection (WoP row 105 = bo + Wo@bv).
            onesbig = persist.tile([1, H * TB], BF16)
            nc.vector.memset(onesbig[:], 1.0)
            nc.gpsimd.dma_start(
                out=oT[105:106, :, :].rearrange("p h t -> p (h t)"),
                in_=onesbig[:],
            )

            # ---- HAM warmup: back-to-back transposes keep the PE busy
            # from ~5us so the clock gate is at 2.4GHz when qproj starts.
            for _ in range(N_WARM):
                pswm = psProj.tile([P, P], BF16, tag="proj")
                nc.tensor.transpose(pswm[:], idents[:], idents[:])

            with tc.tile_pool(name="acts", bufs=1) as acts:
                kT = acts.tile([P, 8, T], BF16)  # [d_model rows, oc, s]
                vS = acts.tile([P, 8, D], BF16)  # [s rows, sc, d_model]
                qT = acts.tile([P, 8, TB], BF16)

                with (
                    tc.tile_pool(name="psSc", bufs=2, space="PSUM") as psSc,
                    tc.tile_pool(name="psO", bufs=2, space="PSUM") as psO,
                ):
                    with tc.tile_pool(name="win", bufs=1) as win:
                        # DMA in consumption order. sync: hsTq, first two
                        # WqP chunks, then hsT per-(ic,half) chunks (n0
                        # halves first so kproj n0 groups close early).
                        # gpsimd: the rest of WqP, WkP, WvT.
                        hsTqs = win.tile([P, 8, TB], BF16)
                        for ic in range(8):
                            nc.sync.dma_start(
                                hsTqs[:, ic, :],
                                hsTq.rearrange("(ic p) t -> p ic t", p=P)[:, ic, :],
                            )
                        Wq_s = win.tile([P, 8, D], BF16)  # [p, oc, ic*128+j]
                        for oc in range(2):
                            nc.sync.dma_start(Wq_s[:, oc, :], WqP[oc, :, :])
                        for oc in range(2, 8):
                            nc.gpsimd.dma_start(Wq_s[:, oc, :], WqP[oc, :, :])
                        hsTs = win.tile([P, 8, T], BF16)  # [p, ic, s]
                        for n in range(2):
                            for ic in range(8):
                                nc.sync.dma_start(
                                    hsTs[:, ic, n * 512 : (n + 1) * 512],
                                    hsT.rearrange("(ic p) s -> p ic s", p=P)[
                                        :, ic, n * 512 : (n + 1) * 512
                                    ],
                                )
                        Wk_s = win.tile([P, 8, D], BF16)  # [p, oc, ic*128+j]
                        for oc in range(8):
                            nc.gpsimd.dma_start(Wk_s[:, oc, :], WkP[oc, :, :])
                        Wv_s = win.tile([P, 8, D], BF16)  # [p, ic, o]
                        for ic in range(8):
                            nc.gpsimd.dma_start(
                                Wv_s[:, ic, :],
                                WvT.rearrange("(ic p) o -> p ic o", p=P)[:, ic, :],
                            )

                        # ---- q projection ----
                        for oc in range(8):
                            ps = psProj.tile([P, 512], F32, tag="proj")
                            for ic in range(8):
                                nc.tensor.matmul(
                                    ps[:, :TB],
                                    lhsT=Wq_s[:, oc, ic * P : (ic + 1) * P],
                                    rhs=hsTqs[:, ic, :],
                                    start=(ic == 0),
                                    stop=(ic == 7),
                                    skip_group_check=True,
                                )
                            nc.vector.tensor_copy(out=qT[:, oc, :], in_=ps[:, :TB])

                        # ---- k projection (n-outer so n0 groups only need
                        # the n0 halves of hsT) ----
                        for n in range(2):
                            for oc in range(8):
                                ps = psProj.tile([P, 512], F32, tag="proj")
                                for ic in range(8):
                                    nc.tensor.matmul(
                                        ps[:],
                                        lhsT=Wk_s[:, oc, ic * P : (ic + 1) * P],
                                        rhs=hsTs[:, ic, n * 512 : (n + 1) * 512],
                                        start=(ic == 0),
                                        stop=(ic == 7),
                                        skip_group_check=True,
                                    )
                                nc.vector.tensor_copy(
                                    out=kT[:, oc, n * 512 : (n + 1) * 512], in_=ps[:]
                                )

                        # ---- scores+exp interleaved with v projection ----
                        # 4 score chunks land in one [128,1024] psum tile ->
                        # one batched exp; two vproj psum groups per head
                        # keep the PE streaming while the scalar engine
                        # drains the score tiles.
                        vgroups = [(sc, n) for sc in range(8) for n in range(2)]
                        for h in range(H):
                            base = (h % 2) * 64
                            oc = h // 2
                            for g4 in range(2):
                                ps = psSc.tile([P, 1024], F32, tag="scoresT")
                                for k4 in range(4):
                                    sc = 4 * g4 + k4
                                    nc.tensor.matmul(
                                        ps[:, 256 * k4 : 256 * (k4 + 1)],
                                        lhsT=kT[
                                            base : base + 64, oc, sc * P : (sc + 1) * P
                                        ],
                                        rhs=qT[base : base + 64, oc, :],
                                        start=True,
                                        stop=True,
                                        skip_group_check=True,
                                    )
                                if with_bias:
                                    for k4 in range(4):
                                        sc = 4 * g4 + k4
                                        nc.scalar.activation(
                                            AT[:, sc, h, :],
                                            ps[:, 256 * k4 : 256 * (k4 + 1)],
                                            mybir.ActivationFunctionType.Exp,
                                            bias=bqks[:, sc, h : h + 1],
                                        )
                                else:
                                    nc.scalar.activation(
                                        AT[:, 4 * g4 : 4 * g4 + 4, h, :],
                                        ps[:],
                                        mybir.ActivationFunctionType.Exp,
                                    )
                            for sc, n in vgroups[2 * h : 2 * h + 2]:
                                ps = psProj.tile([P, 512], F32, tag="proj")
                                for ic in range(8):
                                    nc.tensor.matmul(
                                        ps[:],
                                        lhsT=hsTs[:, ic, sc * P : (sc + 1) * P],
                                        rhs=Wv_s[:, ic, n * 512 : (n + 1) * 512],
                                        start=(ic == 0),
                                        stop=(ic == 7),
                                        skip_group_check=True,
                                    )
                                nc.vector.tensor_copy(
                                    out=vS[:, sc, n * 512 : (n + 1) * 512], in_=ps[:]
                                )

                    # ---- out1: attn @ v per head ----
                    for h in range(H):
                        pso = psO.tile([64, TB], F32, tag="out1T")
                        for sc in range(8):
                            nc.tensor.matmul(
                                pso[:],
                                lhsT=vS[:, sc, h * DH : (h + 1) * DH],
                                rhs=AT[:, sc, h, :],
                                start=(sc == 0),
                                stop=(sc == 7),
                                skip_group_check=True,
                            )
                        nc.vector.tensor_copy(out=oT[0:64, h, :], in_=pso[:])

            # acts freed; psSc/psO banks freed for psW/psL.
            with (
                tc.tile_pool(name="late", bufs=1) as late,
                tc.tile_pool(name="psW", bufs=2, space="PSUM") as psW,
                tc.tile_pool(name="psL", bufs=2, space="PSUM") as psL,
            ):
                WoPs = late.tile([P, H, D], BF16)
                nc.scalar.dma_start(WoPs[:], WoP.rearrange("h p o -> p h o"))
                WsbE = late.tile([P, NG, 64], BF16)  # even-sc W, j in 0:42
                WsbO = late.tile([P, NG, 64], BF16)  # odd-sc W
                Rzm = late.tile([P, TB], F32)
                nc.vector.memset(Rzm[:], 0.0)
                outsb = late.tile([P, 2, D], F32)

                # ---- W histogram matmuls ----
                # t = grp*4 + c; strip c: 32-col weight = (2 sc x 16 h) of t,
                # rhs = [O(sc even) | O(sc odd)] fp8. psw rows 32c+16ksc+h:
                # ksc=0 rows valid at cols 0:42, ksc=1 at 42:84.
                for grp in range(NG):
                    ohs = ohp.tile([P, 4, 2 * NJ * 4], FP8, tag="oh")
                    qeng = nc.sync if grp % 2 else nc.gpsimd
                    qeng.dma_start(
                        ohs[:].rearrange("p a b -> p (a b)"), OH[grp, :, :]
                    )
                    psw = psW.tile([P, 2 * NJ], F32, tag="wps")
                    for sp in range(4):
                        for c in range(4):
                            nc.tensor.matmul(
                                psw[32 * c : 32 * c + 32, :],
                                lhsT=AT[
                                    :, 2 * sp : 2 * sp + 2, :, grp * 4 + c
                                ].rearrange("p a h -> p (a h)"),
                                rhs=ohs[:, sp, 2 * NJ * c : 2 * NJ * (c + 1)],
                                start=(sp == 0),
                                stop=(sp == 3),
                                tile_position=(0, 32 * c),
                                skip_group_check=True,
                            )
                    # ones column 41 = this row's half-Z
                    nc.vector.tensor_copy(out=Zh[:, grp : grp + 1], in_=psw[:, 41:42])
                    nc.vector.tensor_copy(out=WsbE[:, grp, 0:NJ], in_=psw[:, 0:NJ])
                    nc.vector.tensor_copy(
                        out=WsbO[:, grp, 0:NJ], in_=psw[:, NJ : 2 * NJ]
                    )
                    if grp % 2 == 1:
                        gp = grp // 2
                        psTE = psL.tile([P, P], BF16, tag="tE")
                        nc.tensor.transpose(
                            psTE[:],
                            WsbE[:, grp - 1 : grp + 1, :].rearrange("p a x -> p (a x)"),
                            idents[:],
                        )
                        psTO = psL.tile([P, P], BF16, tag="tO")
                        nc.tensor.transpose(
                            psTO[:],
                            WsbO[:, grp - 1 : grp + 1, :].rearrange("p a x -> p (a x)"),
                            idents[:],
                        )
                        # DVE reads one psum input per op: copy even, add odd.
                        for bb in range(2):
                            g = 2 * gp + bb
                            srcE = psTE[64 * bb : 64 * bb + 41, :].rearrange(
                                "p (c k h) -> p h k c", c=4, k=2
                            )[:, :, 0, :]
                            srcO = psTO[64 * bb : 64 * bb + 41, :].rearrange(
                                "p (c k h) -> p h k c", c=4, k=2
                            )[:, :, 1, :]
                            nc.vector.tensor_copy(
                                out=oT[64:105, :, g * 4 : g * 4 + 4], in_=srcE
                            )
                            nc.vector.tensor_tensor(
                                out=oT[64:105, :, g * 4 : g * 4 + 4],
                                in0=oT[64:105, :, g * 4 : g * 4 + 4],
                                in1=srcO,
                                op=mybir.AluOpType.add,
                            )

                # ---- Z fold + 1/Z broadcast + normalize ----
                psz = psW.tile([P, 2 * NJ], F32, tag="wps")
                nc.tensor.matmul(
                    psz[:, 0:NG],
                    lhsT=folds[:],
                    rhs=Zh[:],
                    start=True,
                    stop=True,
                    skip_group_check=True,
                )
                nc.vector.reciprocal(out=RZ[:], in_=psz[:, 0:NG])
                # Rzm[p, t] = 1/Z[t, h(p)] at valid rows; junk rows stay 0.
                for c in range(4):
                    nc.vector.tensor_tensor(
                        out=Rzm[32 * c : 32 * c + 16, :].rearrange(
                            "p (g u) -> p g u", g=NG
                        ),
                        in0=RZ[32 * c : 32 * c + 16, :, None].to_broadcast(
                            [16, NG, 4]
                        ),
                        in1=tmasks[32 * c : 32 * c + 16, :].rearrange(
                            "p (g u) -> p g u", g=NG
                        ),
                        op=mybir.AluOpType.mult,
                    )
                for h in range(H):
                    psb = psL.tile([105, TB], F32, tag="tE")
                    nc.tensor.matmul(
                        psb[:],
                        lhsT=hsels[:, h, :],
                        rhs=Rzm[:],
                        start=True,
                        stop=True,
                        skip_group_check=True,
                    )
                    nc.vector.tensor_tensor(
                        out=oT[0:105, h, :],
                        in0=oT[0:105, h, :],
                        in1=psb[:],
                        op=mybir.AluOpType.mult,
                    )

                # ---- fused output projection, DMA'd per chunk ----
                outv = out.rearrange("(tc p) o -> p tc o", p=P)
                for tc_i in range(2):
                    for ocj in range(2):
                        pso = psProj.tile([P, 512], F32, tag="proj")
                        for h in range(H):
                            nc.tensor.matmul(
                                pso[:],
                                lhsT=oT[:, h, tc_i * P : (tc_i + 1) * P],
                                rhs=WoPs[:, h, ocj * 512 : (ocj + 1) * 512],
                                start=(h == 0),
                                stop=(h == H - 1),
                                skip_group_check=True,
                            )
                        nc.vector.tensor_copy(
                            out=outsb[:, tc_i, ocj * 512 : (ocj + 1) * 512], in_=pso[:]
                        )
                        nc.sync.dma_start(
                            out=outv[:, tc_i, ocj * 512 : (ocj + 1) * 512],
                            in_=outsb[:, tc_i, ocj * 512 : (ocj + 1) * 512],
                        )

    nc.compile()
    return nc


_NC = {}
_last_in_maps = None


def _get_nc(with_bias: bool = False):
    if with_bias not in _NC:
        _NC[with_bias] = build_bass(with_bias)
    return _NC[with_bias]


def _prep_in_maps(hidden_states, relation_inputs, Wq, bq, Wk, bk, Wv, bv, Wo, bo, rel_emb):
    hidden_states = np.asarray(hidden_states, dtype=np.float32)
    relation_inputs = np.asarray(relation_inputs)
    scale = DH ** -0.5
    bf = ml_dtypes.bfloat16
    fp8np = mybir.dt.np(FP8)

    def per_oc(wT):  # [in, out] -> [oc, p, ic*128+j]
        return np.ascontiguousarray(
            wT.reshape(8, P, 8, P).transpose(2, 1, 0, 3).reshape(8, P, D)
        )

    WqP = per_oc(np.asarray(Wq, np.float32).T * scale).astype(bf)
    WkP = per_oc(np.asarray(Wk, np.float32).T).astype(bf)
    WvT = np.asarray(Wv, np.float32).T.astype(bf)
    Wo = np.asarray(Wo, np.float32)
    E = np.asarray(rel_emb, np.float32)

    WoP = np.zeros((H, P, D), np.float32)
    for h in range(H):
        Wo_h = Wo[:, h * DH : (h + 1) * DH]  # [D, 64]
        WoP[h, 0:64, :] = Wo_h.T
        WoP[h, 64:105, :] = E @ Wo_h.T
    WoP[0, 105, :] = np.asarray(bo, np.float32) + Wo @ np.asarray(bv, np.float32)
    WoP = WoP.astype(bf)

    # bqk[s, h] = k_h[s] . (bq_h * scale) = (hs_b @ Wk_h.T @ bq_h*scale)[s]
    bqs = np.asarray(bq, np.float32) * scale
    with_bias = bool(np.any(bqs))
    wb = np.zeros((D, H), np.float32)
    if with_bias:
        for h in range(H):
            wb[:, h] = np.asarray(Wk, np.float32)[h * DH : (h + 1) * DH, :].T @ bqs[
                h * DH : (h + 1) * DH
            ]

    # helpers: p = 32c + 16ksc + hh in the W-phase psum layout
    pidx = np.arange(P)
    c_p, h_p = pidx // 32, pidx % 32
    hsel_np = (h_p[:, None] == np.arange(H)[None, :]).astype(np.float32)
    hsel_np = np.repeat(hsel_np[:, :, None], 105, axis=2).reshape(P, H * 105)
    tt = np.arange(TB)
    tmask_np = (tt[None, :] % 4 == c_p[:, None]).astype(np.float32)
    fold_np = np.zeros((P, P), np.float32)
    fold_np[pidx, 32 * (pidx // 32) + (pidx % 16)] = 1.0
    ident_np = np.eye(P, dtype=np.float32).astype(bf)

    in_maps = []
    for core in range(N_CORES):
        b, tb = core // 4, core % 4
        hs_b = hidden_states[b]
        hsT_b = np.ascontiguousarray(hs_b.T).astype(bf)
        hsTq = np.ascontiguousarray(hs_b.T[:, tb * TB : (tb + 1) * TB]).astype(bf)
        bqk_c = (hs_b @ wb).astype(np.float32)

        # one-hot(+ones) blocks: OH[g, s', (sp, c, ke, j)]; t = g*4 + c,
        # sc = 2*sp + ke
        rc = np.asarray(relation_inputs[b, tb * TB : (tb + 1) * TB, :])  # [256,1024]
        oh = np.zeros((TB * T, NJ), np.uint8)
        oh[np.arange(TB * T), rc.ravel()] = 1
        oh = oh.reshape(TB, T, NJ)
        oh[:, :, 41] = 1
        oh = oh.reshape(NG, 4, 4, 2, P, NJ).transpose(0, 4, 2, 1, 3, 5)
        oh = np.ascontiguousarray(oh).reshape(NG, P, 4 * 4 * 2 * NJ).astype(fp8np)

        in_maps.append(
            dict(
                hsT=hsT_b,
                hsTq=hsTq,
                WqP=WqP,
                WkP=WkP,
                WvT=WvT,
                WoP=WoP,
                bqk=bqk_c,
                OH=oh,
                hsel=hsel_np,
                tmask=tmask_np,
                fold=fold_np,
                ident=ident_np,
            )
        )
    return in_maps, with_bias


def kernel(hidden_states, relation_inputs, Wq, bq, Wk, bk, Wv, bv, Wo, bo, rel_emb):
    global _last_in_maps
    in_maps, with_bias = _prep_in_maps(
        hidden_states, relation_inputs, Wq, bq, Wk, bk, Wv, bv, Wo, bo, rel_emb
    )
    _last_in_maps = in_maps
    nc = _get_nc(with_bias)
    res = bass_utils.run_bass_kernel_spmd(nc, in_maps, core_ids=list(range(N_CORES)))
    outs = [np.asarray(r["out"], np.float32) for r in res.results]
    full = np.empty((B, T, D), np.float32)
    for core in range(N_CORES):
        b, tb = core // 4, core % 4
        full[b, tb * TB : (tb + 1) * TB, :] = outs[core]
    return full
